# revision 27
# baseline (speedup 1.0000x reference)
"""OIM loss with circular queue — Trainium2 Bass kernel (8 NeuronCores).

Strategy
--------
The output is a scalar:  loss = mean_b [ logsumexp_{q in good}(30*cos(x_b, e_q))
                                         - 30*cos(x_b, e_{xe_b}) ]
where e is the circular queue after the (sequential, data-dependent) update.

All O(B*D + Q*D) bookkeeping and reshaping runs on the host: the integer
queue update, the per-pid masked means + queue-window write (4 MFLOP), the
rotation of the queue so the window is always core 0 / cols [0,U), zeroing
of invalidated slots (their exact n_bad * exp(-M) contribution is
subtracted from the device sums), fp8 quantization, and the d-major
DoubleRow layouts.  The O(B*Q*D) = 68.7 GFLOP logits matmul and the
B*Q = 67M-element exp/log-sum-exp run on the 8 cores, tensor-parallel
over Q (2048 queue rows per core):

  per m-tile of 128 rows: 8 fp8 DoubleRow matmuls accumulate the [128,2048]
  logits into one 4-bank PSUM tile; ONE exp activation per tile applies the
  per-row scale 30/||x_b|| (input normalization folded into the activation
  scale) and bias -M; row-sums of the bf16 exp output run on DVE (2-byte
  fast mode), keeping the ACT engine saturated with pure 2048-wide exps —
  the throughput floor of the whole kernel; the target dot is gathered
  from cols [0,256) of PSUM with a one-hot DVE op (the window holds every
  batch pid's embedding) and scaled on the host.

The host adds the 8 partial sums, applies the zero-row correction, takes
log and means.
"""

import os
import sys

import numpy as np

for _p in ("/opt/trn_rl_repo", "/root/.axon_site/_ro/trn_rl_repo"):
    if os.path.isdir(_p) and _p not in sys.path:
        sys.path.insert(0, _p)

import ml_dtypes

BF16 = ml_dtypes.bfloat16
FP8 = ml_dtypes.float8_e4m3

B, D, Q, U = 4096, 512, 16384, 256
N_CORES = 8
QS = Q // N_CORES          # queue rows per core
OIM_SCALAR = 30.0
IGNORE = -1
MT = B // 128              # 32 b-tiles
KD = D // 128              # 4 contraction chunks
KP = KD // 2               # 2 chunk pairs (DoubleRow)
NQ = QS // 512             # 4 matmul n-chunks per core

_PROG_CACHE = {}


def _build_program(M: float):
    """Emit + schedule + compile the (SPMD, identical on all cores) program."""
    import concourse.bacc as bacc
    import concourse.tile as tile
    from concourse import mybir
    from concourse.masks import make_identity

    f32 = mybir.dt.float32
    bf16 = mybir.dt.bfloat16
    fp8 = mybir.dt.float8e4
    AF = mybir.ActivationFunctionType
    OP = mybir.AluOpType
    DR = mybir.MatmulPerfMode.DoubleRow

    nc = bacc.Bacc("TRN2", target_bir_lowering=False, debug=False,
                   num_devices=N_CORES)

    # consts columns: scl | widx | iota | tick
    CW = MT + MT + U + 4
    xt_d = nc.dram_tensor("xt8", [128, KP, 2, B], fp8, kind="ExternalInput").ap()
    embt_d = nc.dram_tensor("embt8", [128, KP, 2, QS], fp8,
                            kind="ExternalInput").ap()
    consts_d = nc.dram_tensor("consts", [128, CW], f32,
                              kind="ExternalInput").ap()
    sume_d = nc.dram_tensor("sume", [128, MT], f32, kind="ExternalOutput").ap()
    tco_d = nc.dram_tensor("tco", [128, MT], f32, kind="ExternalOutput").ap()
    tock_d = nc.dram_tensor("tock", [128, 4], f32, kind="ExternalOutput").ap()

    with tile.TileContext(nc) as tc:
        with (
            tc.tile_pool(name="singles", bufs=1) as singles,
            tc.tile_pool(name="work", bufs=4) as work,
            tc.tile_pool(name="small", bufs=4) as small,
        ):
            # one merged consts DMA:  scl | widx | iota | tick
            consts = singles.tile([128, CW], f32)
            nc.sync.dma_start(out=consts, in_=consts_d)
            scl = consts[:, 0:MT]
            widx = consts[:, MT:2 * MT]
            iotab = consts[:, 2 * MT:2 * MT + U]
            tickt = consts[:, 2 * MT + U:CW]

            biasM = singles.tile([128, 1], f32)
            nc.vector.memset(biasM, -M)
            identb = singles.tile([128, 128], bf16)
            make_identity(nc, identb)

            # preload the Exp table off the critical path (the only ACT func)
            warm = small.tile([128, 1], f32, tag="warm")
            nc.vector.memset(warm, 1.0)
            warm2 = small.tile([128, 1], f32, tag="warm2")
            nc.scalar.activation(out=warm2, in_=warm, func=AF.Exp)

            ssb = singles.tile([128, MT], f32)    # sum-exp out collector
            praw = singles.tile([128, MT], f32)   # exp(target logit - M) out
            embT = singles.tile([128, KP, 2, QS], fp8)
            xT = singles.tile([128, KP, 2, B], fp8)

            # input streams: the first exp is gated by the FULL embT (every
            # n-block of psm m=0), so embT streams first and uninterrupted;
            # a small first xT chunk lets m=0..1 matmul, bulk follows
            nc.sync.dma_start(out=embT, in_=embt_d)
            nc.sync.dma_start(out=xT[:, :, :, 0:B // 16],
                              in_=xt_d[:, :, :, 0:B // 16])
            nc.sync.dma_start(out=xT[:, :, :, B // 16:B // 8],
                              in_=xt_d[:, :, :, B // 16:B // 8])
            for c in range(1, 8):
                nc.sync.dma_start(out=xT[:, :, :, c * B // 8:(c + 1) * B // 8],
                                  in_=xt_d[:, :, :, c * B // 8:(c + 1) * B // 8])

            # tick/tock passthrough (no compute dependency; queued last so
            # its descriptor slot never delays an input chunk)
            nc.sync.dma_start(out=tock_d, in_=tickt)

            # spin the PE p-state up with throwaway transposes so the first
            # logits matmuls run at full clock the moment their DMAs land
            with tc.tile_pool(name="psum_w", bufs=1, space="PSUM") as psum_w:
                wps = psum_w.tile([128, 128], bf16, tag="wps")
                for _ in range(34):
                    nc.tensor.transpose(wps, identb, identb)

            # ---------------- logits + exp + row sums ----------------
            with tc.tile_pool(name="psum_m", bufs=2, space="PSUM") as psum_m:
                for m in range(MT):
                    psm = psum_m.tile([128, NQ * 512], f32, tag="psm")
                    for n in range(NQ - 1, -1, -1):
                        for j in range(KP):
                            nc.tensor.matmul(
                                psm[:, n * 512:(n + 1) * 512],
                                xT[:, j, :, m * 128:(m + 1) * 128],
                                embT[:, j, :, n * 512:(n + 1) * 512],
                                start=(j == 0), stop=(j == KP - 1),
                                perf_mode=DR)
                    # target: gather the raw dot of window col widx_b from
                    # PSUM; the host applies the 30/||x_b|| scale
                    scr = work.tile([128, U], f32, tag="scr")
                    nc.vector.scalar_tensor_tensor(
                        out=scr, in0=iotab, scalar=widx[:, m:m + 1],
                        in1=psm[:, 0:U], op0=OP.is_equal, op1=OP.mult,
                        accum_out=praw[:, m:m + 1])
                    expt = work.tile([128, NQ * 512], bf16, tag="expt", bufs=4)
                    if m == MT - 1:
                        # last tile: let ACT accumulate the row-sum itself so
                        # the result exists the moment the exp retires
                        nc.scalar.activation(out=expt, in_=psm, func=AF.Exp,
                                             bias=biasM, scale=scl[:, m:m + 1],
                                             accum_out=ssb[:, m:m + 1])
                        nc.sync.dma_start(out=tco_d, in_=praw)
                    else:
                        nc.scalar.activation(out=expt, in_=psm, func=AF.Exp,
                                             bias=biasM, scale=scl[:, m:m + 1])
                        # row-sum off the ACT engine (bf16 keeps DVE fast)
                        dummy = work.tile([128, NQ * 512], bf16, tag="dumm",
                                          bufs=2)
                        nc.vector.tensor_scalar(out=dummy, in0=expt,
                                                scalar1=1.0, scalar2=0.0,
                                                op0=OP.mult, op1=OP.add,
                                                accum_out=ssb[:, m:m + 1])
                    if m == MT - 2:      # all but the last column go out early
                        nc.sync.dma_start(out=sume_d[:, 0:MT - 1],
                                          in_=ssb[:, 0:MT - 1])

            nc.sync.dma_start(out=sume_d[:, MT - 1:MT],
                              in_=ssb[:, MT - 1:MT])

    nc.compile()
    return nc


def _host_bookkeeping(labels, label_cq, header_cq):
    """Mirror the reference's integer-only queue-update semantics."""
    labels = np.asarray(labels).astype(np.int64)
    lab = np.asarray(label_cq).astype(np.int64).copy()
    h0 = int(np.asarray(header_cq))

    # jnp.unique(labels, size=U): sorted unique, padded with the minimum
    uq = np.unique(labels)
    if uq.size < U:
        uniq = np.concatenate([uq, np.full(U - uq.size, uq.min(), np.int64)])
    else:
        uniq = uq[:U]

    emb_src = np.full(Q, -1, np.int64)   # >=0: row u of uniq means; -1: original
    h = h0 % Q
    for u in range(U):
        y = uniq[u]
        m = lab == y
        i = int(np.argmax(m)) if m.any() else 0
        inval = bool(m.any()) and (i != h)
        emb_src[h] = u
        lab[h] = y
        if inval:
            lab[i] = IGNORE
        h = (h + 1) % Q

    good = lab != IGNORE
    goodidx = np.flatnonzero(good)
    gl = lab[goodidx]
    vals, first = np.unique(gl, return_index=True)
    pos = np.searchsorted(vals, labels)
    assert np.all(vals[np.clip(pos, 0, vals.size - 1)] == labels), \
        "batch label missing from queue"
    xe = goodidx[first[pos]]
    return uniq, emb_src, good, xe, h0


def _prepare(inputs, labels, emb_cq, label_cq, header_cq):
    """Host bookkeeping -> (M, per-core input maps, extras, correction)."""
    inputs = np.ascontiguousarray(np.asarray(inputs, np.float32))
    emb_cq = np.ascontiguousarray(np.asarray(emb_cq, np.float32))
    labels = np.asarray(labels)

    uniq, emb_src, good, xe, h0 = _host_bookkeeping(labels, label_cq,
                                                    header_cq)

    # safe upper bound for any logit: 30 * max row norm (+fp8 slack)
    max_nrm = float(np.sqrt((emb_cq.astype(np.float64) ** 2).sum(axis=1).max()))
    M = OIM_SCALAR * max(1.0, max_nrm) * 1.10

    w_idx = emb_src[xe].astype(np.float64)        # -1 for non-window targets
    extra = np.flatnonzero(w_idx < 0)             # handled on host (rare/none)

    # per-pid masked means -> normalized window embeddings (4 MFLOP)
    x64 = inputs.astype(np.float64)
    m_u = (uniq[:, None] == labels[None, :].astype(np.int64))
    uniq_emb = (m_u.astype(np.float64) @ x64) / m_u.sum(axis=1, keepdims=True)
    uniq_emb /= np.maximum(
        np.linalg.norm(uniq_emb, axis=1, keepdims=True), 1e-12)

    # rotate the queue so the window is slots [0, U) -> core 0, cols [0, U)
    rot = (h0 + np.arange(Q)) % Q
    emb_rot = emb_cq[rot].copy()
    good_rot = good[rot]
    src_rot = emb_src[rot]
    assert np.all(src_rot[:U] == np.arange(U)) and np.all(src_rot[U:] < 0)
    emb_rot[:U] = uniq_emb                        # the queue-window write
    # zero invalidated slots; host subtracts their exp(0-M) later
    zero_rows = np.zeros(Q, bool)
    zero_rows[U:] = ~good_rot[U:]
    emb_rot[zero_rows] = 0.0
    n_bad = int(zero_rows.sum())

    def dmajor8(a):  # [R, D] f32 -> [128, KP, 2, R] fp8 (DoubleRow pairs)
        r = a.shape[0]
        return np.ascontiguousarray(
            a.T.reshape(KP, 2, 128, r).transpose(2, 0, 1, 3).astype(FP8))

    # 30 / ||x_b|| (exp scale; folds input normalization)
    nrm = np.sqrt((x64 ** 2).sum(axis=1))
    sclv = OIM_SCALAR / np.maximum(nrm, 1e-12)

    # consts columns: scl | widx | iota | tick
    consts = np.concatenate([
        sclv.reshape(MT, 128).T.astype(np.float32),
        w_idx.reshape(MT, 128).T.astype(np.float32),
        np.broadcast_to(np.arange(U, dtype=np.float32), (128, U)),
        np.zeros((128, 4), np.float32),
    ], axis=1)

    base = {
        "xt8": dmajor8(inputs),
        "consts": np.ascontiguousarray(consts),
    }
    in_maps = []
    for c in range(N_CORES):
        in_maps.append({
            **base,
            "embt8": dmajor8(emb_rot[c * QS:(c + 1) * QS].astype(np.float32)),
        })
    return M, in_maps, extra, xe, n_bad, sclv


def _combine(res_list, M, extra, xe, n_bad, sclv, inputs, emb_cq):
    """Unshard / combine per-core partials into the scalar loss."""
    S = np.zeros(B, np.float64)
    for r in res_list:
        S += r["sume"].astype(np.float64).T.reshape(B)
    S -= n_bad * np.exp(-float(M))                # zeroed rows' exp(0 - M)
    t30 = res_list[0]["tco"].astype(np.float64).T.reshape(B) * sclv

    if extra.size:  # targets pointing at original (non-window) queue rows
        xb = np.asarray(inputs, np.float64)[extra]
        xb /= np.maximum(np.linalg.norm(xb, axis=1, keepdims=True), 1e-12)
        eb = np.asarray(emb_cq, np.float64)[xe[extra]]
        t30[extra] = OIM_SCALAR * (xb * eb).sum(axis=1)

    loss = np.mean(M + np.log(S) - t30)
    return np.array(loss, dtype=np.float32)


def kernel(inputs, labels, emb_cq, label_cq, age_cq, header_cq):
    from concourse.bass_utils import run_bass_kernel_spmd

    M, in_maps, extra, xe, n_bad, sclv = _prepare(
        inputs, labels, emb_cq, label_cq, header_cq)

    key = round(M, 9)
    if key not in _PROG_CACHE:
        _PROG_CACHE[key] = _build_program(M)
    nc = _PROG_CACHE[key]

    res = run_bass_kernel_spmd(nc, in_maps, core_ids=list(range(N_CORES)))
    return _combine(res.results, M, extra, xe, n_bad, sclv, inputs, emb_cq)


# revision 28
# speedup vs baseline: 1.0013x; 1.0013x over previous
"""OIM loss with circular queue — Trainium2 Bass kernel (8 NeuronCores).

Strategy
--------
The output is a scalar:  loss = mean_b [ logsumexp_{q in good}(30*cos(x_b, e_q))
                                         - 30*cos(x_b, e_{xe_b}) ]
where e is the circular queue after the (sequential, data-dependent) update.

All O(B*D + Q*D) bookkeeping and reshaping runs on the host: the integer
queue update, the per-pid masked means + queue-window write (4 MFLOP), the
rotation of the queue so the window is always core 0 / cols [0,U), zeroing
of invalidated slots (their exact n_bad * exp(-M) contribution is
subtracted from the device sums), fp8 quantization, and the d-major
DoubleRow layouts.  The O(B*Q*D) = 68.7 GFLOP logits matmul and the
B*Q = 67M-element exp/log-sum-exp run on the 8 cores, tensor-parallel
over Q (2048 queue rows per core):

  per m-tile of 128 rows: 8 fp8 DoubleRow matmuls accumulate the [128,2048]
  logits into one 4-bank PSUM tile; ONE exp activation per tile applies the
  per-row scale 30/||x_b|| (input normalization folded into the activation
  scale) and bias -M; row-sums of the bf16 exp output run on DVE (2-byte
  fast mode), keeping the ACT engine saturated with pure 2048-wide exps —
  the throughput floor of the whole kernel; the target dot is gathered
  from cols [0,256) of PSUM with a one-hot DVE op (the window holds every
  batch pid's embedding) and scaled on the host.

The host adds the 8 partial sums, applies the zero-row correction, takes
log and means.
"""

import os
import sys

import numpy as np

for _p in ("/opt/trn_rl_repo", "/root/.axon_site/_ro/trn_rl_repo"):
    if os.path.isdir(_p) and _p not in sys.path:
        sys.path.insert(0, _p)

import ml_dtypes

BF16 = ml_dtypes.bfloat16
FP8 = ml_dtypes.float8_e4m3

B, D, Q, U = 4096, 512, 16384, 256
N_CORES = 8
QS = Q // N_CORES          # queue rows per core
OIM_SCALAR = 30.0
IGNORE = -1
MT = B // 128              # 32 b-tiles
KD = D // 128              # 4 contraction chunks
KP = KD // 2               # 2 chunk pairs (DoubleRow)
NQ = QS // 512             # 4 matmul n-chunks per core

_PROG_CACHE = {}


def _build_program(M: float):
    """Emit + schedule + compile the (SPMD, identical on all cores) program."""
    import concourse.bacc as bacc
    import concourse.tile as tile
    from concourse import mybir
    from concourse.masks import make_identity

    f32 = mybir.dt.float32
    bf16 = mybir.dt.bfloat16
    fp8 = mybir.dt.float8e4
    AF = mybir.ActivationFunctionType
    OP = mybir.AluOpType
    DR = mybir.MatmulPerfMode.DoubleRow

    nc = bacc.Bacc("TRN2", target_bir_lowering=False, debug=False,
                   num_devices=N_CORES)

    # consts columns: scl | widx | iota | tick
    CW = MT + MT + U + 4
    xt_d = nc.dram_tensor("xt8", [128, KP, 2, B], fp8, kind="ExternalInput").ap()
    embt_d = nc.dram_tensor("embt8", [128, KP, 2, QS], fp8,
                            kind="ExternalInput").ap()
    consts_d = nc.dram_tensor("consts", [128, CW], f32,
                              kind="ExternalInput").ap()
    sume_d = nc.dram_tensor("sume", [128, MT], f32, kind="ExternalOutput").ap()
    tco_d = nc.dram_tensor("tco", [128, MT], f32, kind="ExternalOutput").ap()
    tock_d = nc.dram_tensor("tock", [128, 4], f32, kind="ExternalOutput").ap()

    with tile.TileContext(nc) as tc:
        with (
            tc.tile_pool(name="singles", bufs=1) as singles,
            tc.tile_pool(name="work", bufs=4) as work,
            tc.tile_pool(name="small", bufs=4) as small,
        ):
            # one merged consts DMA:  scl | widx | iota | tick
            consts = singles.tile([128, CW], f32)
            nc.sync.dma_start(out=consts, in_=consts_d)
            scl = consts[:, 0:MT]
            widx = consts[:, MT:2 * MT]
            iotab = consts[:, 2 * MT:2 * MT + U]
            tickt = consts[:, 2 * MT + U:CW]

            biasM = singles.tile([128, 1], f32)
            nc.vector.memset(biasM, -M)
            identb = singles.tile([128, 128], bf16)
            make_identity(nc, identb)

            # preload the Exp table off the critical path (the only ACT func)
            warm = small.tile([128, 1], f32, tag="warm")
            nc.vector.memset(warm, 1.0)
            warm2 = small.tile([128, 1], f32, tag="warm2")
            nc.scalar.activation(out=warm2, in_=warm, func=AF.Exp)

            ssb = singles.tile([128, MT], f32)    # sum-exp out collector
            praw = singles.tile([128, MT], f32)   # exp(target logit - M) out
            embT = singles.tile([128, KP, 2, QS], fp8)
            xT = singles.tile([128, KP, 2, B], fp8)

            # tick/tock passthrough early (no compute dependency)
            nc.sync.dma_start(out=tock_d, in_=tickt)

            # input streams in consumption order (n runs 3..0, m runs 0..31):
            # fine chunks up front so the first PSUM tile completes early,
            # coarse chunks after
            nc.sync.dma_start(out=embT[:, :, :, 3 * QS // 4:QS],
                              in_=embt_d[:, :, :, 3 * QS // 4:QS])
            nc.sync.dma_start(out=xT[:, :, :, 0:B // 8],
                              in_=xt_d[:, :, :, 0:B // 8])
            for c in (2, 1, 0):
                nc.sync.dma_start(
                    out=embT[:, :, :, c * QS // 4:(c + 1) * QS // 4],
                    in_=embt_d[:, :, :, c * QS // 4:(c + 1) * QS // 4])
            for c in range(1, 8):
                nc.sync.dma_start(out=xT[:, :, :, c * B // 8:(c + 1) * B // 8],
                                  in_=xt_d[:, :, :, c * B // 8:(c + 1) * B // 8])

            # spin the PE p-state up with throwaway transposes so the first
            # logits matmuls run at full clock the moment their DMAs land
            with tc.tile_pool(name="psum_w", bufs=1, space="PSUM") as psum_w:
                wps = psum_w.tile([128, 128], bf16, tag="wps")
                for _ in range(34):
                    nc.tensor.transpose(wps, identb, identb)

            # ---------------- logits + exp + row sums ----------------
            with tc.tile_pool(name="psum_m", bufs=2, space="PSUM") as psum_m:
                for m in range(MT):
                    psm = psum_m.tile([128, NQ * 512], f32, tag="psm")
                    for n in range(NQ - 1, -1, -1):
                        for j in range(KP):
                            nc.tensor.matmul(
                                psm[:, n * 512:(n + 1) * 512],
                                xT[:, j, :, m * 128:(m + 1) * 128],
                                embT[:, j, :, n * 512:(n + 1) * 512],
                                start=(j == 0), stop=(j == KP - 1),
                                perf_mode=DR)
                    # target: gather the raw dot of window col widx_b from
                    # PSUM; the host applies the 30/||x_b|| scale
                    scr = work.tile([128, U], f32, tag="scr")
                    nc.vector.scalar_tensor_tensor(
                        out=scr, in0=iotab, scalar=widx[:, m:m + 1],
                        in1=psm[:, 0:U], op0=OP.is_equal, op1=OP.mult,
                        accum_out=praw[:, m:m + 1])
                    expt = work.tile([128, NQ * 512], bf16, tag="expt", bufs=4)
                    if m == MT - 1:
                        # last tile: let ACT accumulate the row-sum itself so
                        # the result exists the moment the exp retires
                        nc.scalar.activation(out=expt, in_=psm, func=AF.Exp,
                                             bias=biasM, scale=scl[:, m:m + 1],
                                             accum_out=ssb[:, m:m + 1])
                        nc.sync.dma_start(out=tco_d, in_=praw)
                    else:
                        nc.scalar.activation(out=expt, in_=psm, func=AF.Exp,
                                             bias=biasM, scale=scl[:, m:m + 1])
                        # row-sum off the ACT engine (bf16 keeps DVE fast)
                        dummy = work.tile([128, NQ * 512], bf16, tag="dumm",
                                          bufs=2)
                        nc.vector.tensor_scalar(out=dummy, in0=expt,
                                                scalar1=1.0, scalar2=0.0,
                                                op0=OP.mult, op1=OP.add,
                                                accum_out=ssb[:, m:m + 1])
                    if m == MT - 2:      # all but the last column go out early
                        nc.sync.dma_start(out=sume_d[:, 0:MT - 1],
                                          in_=ssb[:, 0:MT - 1])

            nc.sync.dma_start(out=sume_d[:, MT - 1:MT],
                              in_=ssb[:, MT - 1:MT])

    nc.compile()
    return nc


def _host_bookkeeping(labels, label_cq, header_cq):
    """Mirror the reference's integer-only queue-update semantics."""
    labels = np.asarray(labels).astype(np.int64)
    lab = np.asarray(label_cq).astype(np.int64).copy()
    h0 = int(np.asarray(header_cq))

    # jnp.unique(labels, size=U): sorted unique, padded with the minimum
    uq = np.unique(labels)
    if uq.size < U:
        uniq = np.concatenate([uq, np.full(U - uq.size, uq.min(), np.int64)])
    else:
        uniq = uq[:U]

    emb_src = np.full(Q, -1, np.int64)   # >=0: row u of uniq means; -1: original
    h = h0 % Q
    for u in range(U):
        y = uniq[u]
        m = lab == y
        i = int(np.argmax(m)) if m.any() else 0
        inval = bool(m.any()) and (i != h)
        emb_src[h] = u
        lab[h] = y
        if inval:
            lab[i] = IGNORE
        h = (h + 1) % Q

    good = lab != IGNORE
    goodidx = np.flatnonzero(good)
    gl = lab[goodidx]
    vals, first = np.unique(gl, return_index=True)
    pos = np.searchsorted(vals, labels)
    assert np.all(vals[np.clip(pos, 0, vals.size - 1)] == labels), \
        "batch label missing from queue"
    xe = goodidx[first[pos]]
    return uniq, emb_src, good, xe, h0


def _prepare(inputs, labels, emb_cq, label_cq, header_cq):
    """Host bookkeeping -> (M, per-core input maps, extras, correction)."""
    inputs = np.ascontiguousarray(np.asarray(inputs, np.float32))
    emb_cq = np.ascontiguousarray(np.asarray(emb_cq, np.float32))
    labels = np.asarray(labels)

    uniq, emb_src, good, xe, h0 = _host_bookkeeping(labels, label_cq,
                                                    header_cq)

    # safe upper bound for any logit: 30 * max row norm (+fp8 slack)
    max_nrm = float(np.sqrt((emb_cq.astype(np.float64) ** 2).sum(axis=1).max()))
    M = OIM_SCALAR * max(1.0, max_nrm) * 1.10

    w_idx = emb_src[xe].astype(np.float64)        # -1 for non-window targets
    extra = np.flatnonzero(w_idx < 0)             # handled on host (rare/none)

    # per-pid masked means -> normalized window embeddings (4 MFLOP)
    x64 = inputs.astype(np.float64)
    m_u = (uniq[:, None] == labels[None, :].astype(np.int64))
    uniq_emb = (m_u.astype(np.float64) @ x64) / m_u.sum(axis=1, keepdims=True)
    uniq_emb /= np.maximum(
        np.linalg.norm(uniq_emb, axis=1, keepdims=True), 1e-12)

    # rotate the queue so the window is slots [0, U) -> core 0, cols [0, U)
    rot = (h0 + np.arange(Q)) % Q
    emb_rot = emb_cq[rot].copy()
    good_rot = good[rot]
    src_rot = emb_src[rot]
    assert np.all(src_rot[:U] == np.arange(U)) and np.all(src_rot[U:] < 0)
    emb_rot[:U] = uniq_emb                        # the queue-window write
    # zero invalidated slots; host subtracts their exp(0-M) later
    zero_rows = np.zeros(Q, bool)
    zero_rows[U:] = ~good_rot[U:]
    emb_rot[zero_rows] = 0.0
    n_bad = int(zero_rows.sum())

    def dmajor8(a):  # [R, D] f32 -> [128, KP, 2, R] fp8 (DoubleRow pairs)
        r = a.shape[0]
        return np.ascontiguousarray(
            a.T.reshape(KP, 2, 128, r).transpose(2, 0, 1, 3).astype(FP8))

    # 30 / ||x_b|| (exp scale; folds input normalization)
    nrm = np.sqrt((x64 ** 2).sum(axis=1))
    sclv = OIM_SCALAR / np.maximum(nrm, 1e-12)

    # consts columns: scl | widx | iota | tick
    consts = np.concatenate([
        sclv.reshape(MT, 128).T.astype(np.float32),
        w_idx.reshape(MT, 128).T.astype(np.float32),
        np.broadcast_to(np.arange(U, dtype=np.float32), (128, U)),
        np.zeros((128, 4), np.float32),
    ], axis=1)

    base = {
        "xt8": dmajor8(inputs),
        "consts": np.ascontiguousarray(consts),
    }
    in_maps = []
    for c in range(N_CORES):
        in_maps.append({
            **base,
            "embt8": dmajor8(emb_rot[c * QS:(c + 1) * QS].astype(np.float32)),
        })
    return M, in_maps, extra, xe, n_bad, sclv


def _combine(res_list, M, extra, xe, n_bad, sclv, inputs, emb_cq):
    """Unshard / combine per-core partials into the scalar loss."""
    S = np.zeros(B, np.float64)
    for r in res_list:
        S += r["sume"].astype(np.float64).T.reshape(B)
    S -= n_bad * np.exp(-float(M))                # zeroed rows' exp(0 - M)
    t30 = res_list[0]["tco"].astype(np.float64).T.reshape(B) * sclv

    if extra.size:  # targets pointing at original (non-window) queue rows
        xb = np.asarray(inputs, np.float64)[extra]
        xb /= np.maximum(np.linalg.norm(xb, axis=1, keepdims=True), 1e-12)
        eb = np.asarray(emb_cq, np.float64)[xe[extra]]
        t30[extra] = OIM_SCALAR * (xb * eb).sum(axis=1)

    loss = np.mean(M + np.log(S) - t30)
    return np.array(loss, dtype=np.float32)


def kernel(inputs, labels, emb_cq, label_cq, age_cq, header_cq):
    from concourse.bass_utils import run_bass_kernel_spmd

    M, in_maps, extra, xe, n_bad, sclv = _prepare(
        inputs, labels, emb_cq, label_cq, header_cq)

    key = round(M, 9)
    if key not in _PROG_CACHE:
        _PROG_CACHE[key] = _build_program(M)
    nc = _PROG_CACHE[key]

    res = run_bass_kernel_spmd(nc, in_maps, core_ids=list(range(N_CORES)))
    return _combine(res.results, M, extra, xe, n_bad, sclv, inputs, emb_cq)
